# revision 12
# baseline (speedup 1.0000x reference)
"""Trainium2 Bass kernel for AttentionWithCache (decode-style attention).

Sharding: tensor-parallel over heads. 32 heads split across 8 cores (4 heads
= 512 features per core). Each core projects q/k/v for its heads, attends
over its slice of the KV cache, and computes a partial o_proj (row-parallel);
the host sums the 8 partial outputs (the all-reduce) and adds bo.

Host-side layout prep (transposes only, no compute):
  - xT   [D, B*S]          x flattened + transposed
  - kT   [B*HL, DH, CACHED] per-core K cache slice, transposed so that the
                            head dim sits on SBUF partitions (scores matmul
                            contracts over DH)
  - v    [B*HL, CACHED, DH] per-core V cache slice (natural layout; ctx
                            matmul contracts over tokens)
  - wqT/wkT/wvT [D, 512], woT [512, D]  transposed weight slices
"""

import sys

if "/opt/trn_rl_repo" not in sys.path:
    sys.path.insert(0, "/opt/trn_rl_repo")

import numpy as np

# Problem constants (hardcoded per contract; kernel.py must be self-contained).
B, S, D, H = 16, 4, 4096, 32
DH = 128
NCORES = 8
HL = H // NCORES          # heads per core = 4
JC = HL * DH              # per-core feature slice = 512
BS = B * S                # 64
CACHED = 4096
SCALE = float(1.0 / np.sqrt(DH))
NEG = -1.0e30

_NC_CACHE = {}


def build_nc(b=B, hl=HL, d=D, cached=CACHED, debug=False, compile_=True):
    """Build (and compile) the per-core Bass program.

    Parametrized over batch/heads/model-dim/cache-len so a scaled-down config
    can run under CoreSim for correctness checks.
    """
    import concourse.bacc as bacc
    import concourse.mybir as mybir
    import concourse.tile as tile

    f32 = mybir.dt.float32
    Exp = mybir.ActivationFunctionType.Exp
    AxX = mybir.AxisListType.X

    bs = b * S
    jc = hl * DH
    dt_ = d // 128        # contraction tiles for projections
    t2n = cached // 128   # KV token tiles
    mcn = d // 512        # o_proj output chunks
    bhn = b * hl
    # pA bank layout (cols): [0, 4*t2n) exp-scores pack, then per-tile den
    # partials, then new-token den partials, then new-token raw scores.
    P3 = 4 * t2n          # new-token raw scores, right after the exp pack
    assert P3 + 4 <= 512

    nc = bacc.Bacc("TRN2", target_bir_lowering=False, debug=debug)

    xT_d = nc.dram_tensor("xT", [d, bs], f32, kind="ExternalInput")
    kT_d = nc.dram_tensor("kT", [bhn, DH, cached], f32, kind="ExternalInput")
    v_d = nc.dram_tensor("v", [bhn, cached, DH], f32, kind="ExternalInput")
    wqT_d = nc.dram_tensor("wqT", [d, jc], f32, kind="ExternalInput")
    wkT_d = nc.dram_tensor("wkT", [d, jc], f32, kind="ExternalInput")
    wvT_d = nc.dram_tensor("wvT", [d, jc], f32, kind="ExternalInput")
    woT_d = nc.dram_tensor("woT", [jc, d], f32, kind="ExternalInput")
    bq_d = nc.dram_tensor("bq", [1, jc], f32, kind="ExternalInput")
    bk_d = nc.dram_tensor("bk", [1, jc], f32, kind="ExternalInput")
    bv_d = nc.dram_tensor("bv", [1, jc], f32, kind="ExternalInput")
    mask_d = nc.dram_tensor("mask", [S, S], f32, kind="ExternalInput")
    ident_d = nc.dram_tensor("ident", [128, 128], f32, kind="ExternalInput")
    out_d = nc.dram_tensor("out", [bs, d], f32, kind="ExternalOutput")

    with tile.TileContext(nc) as tc:
        with (
            tc.tile_pool(name="persist", bufs=1) as pers,
            tc.tile_pool(name="consts", bufs=1) as cst,
        ):
            ones = cst.tile([128, 1], f32)
            nc.vector.memset(ones[:], 1.0)
            onesr = cst.tile([1, 128], f32)
            nc.vector.memset(onesr[:], 1.0)
            mask_sb = cst.tile([S, S], f32)
            nc.sync.dma_start(mask_sb[:], mask_d[:])
            bq_sb = cst.tile([1, jc], f32)
            nc.sync.dma_start(bq_sb[:], bq_d[:])
            bk_sb = cst.tile([1, jc], f32)
            nc.sync.dma_start(bk_sb[:], bk_d[:])
            bv_sb = cst.tile([1, jc], f32)
            nc.sync.dma_start(bv_sb[:], bv_d[:])
            ident_sb = cst.tile([128, 128], f32)
            nc.sync.dma_start(ident_sb[:], ident_d[:])
            xT_sb = cst.tile([128, dt_, bs], f32)
            nc.sync.dma_start(
                xT_sb[:], xT_d[:].rearrange("(a p) i -> p a i", p=128)
            )

            # qT/kTn/vTn: [dh, head*BS + i] with i = 4*b + s
            qT_sb = pers.tile([128, hl * bs], f32)
            kTn_sb = pers.tile([128, hl * bs], f32)
            vTn_sb = pers.tile([128, hl * bs], f32)
            # new-token V rows in natural [s, dh] layout, per (b, head)
            vnat_sb = pers.tile([S, b * hl * DH], f32)
            ctxT_sb = pers.tile([128, hl * bs], f32)

            # ---------------- projections ----------------
            with (
                tc.tile_pool(name="wproj", bufs=4) as wp,
                tc.tile_pool(name="psP", bufs=1, space="PSUM") as psP,
            ):
                p_qk = psP.tile([128, 2 * hl * bs], f32)
                p_v = psP.tile([128, hl * bs], f32)
                for a in range(dt_):
                    wq_t = wp.tile([128, jc], f32, tag="w")
                    nc.sync.dma_start(wq_t[:], wqT_d[128 * a : 128 * (a + 1), :])
                    wk_t = wp.tile([128, jc], f32, tag="w")
                    nc.sync.dma_start(wk_t[:], wkT_d[128 * a : 128 * (a + 1), :])
                    wv_t = wp.tile([128, jc], f32, tag="w")
                    nc.sync.dma_start(wv_t[:], wvT_d[128 * a : 128 * (a + 1), :])
                    xa = xT_sb[:, a, :]
                    for c in range(hl):
                        nc.tensor.matmul(
                            p_qk[:, bs * c : bs * (c + 1)],
                            wq_t[:, 128 * c : 128 * (c + 1)],
                            xa,
                            start=(a == 0 and c == 0),
                            stop=False,
                        )
                        nc.tensor.matmul(
                            p_qk[:, hl * bs + bs * c : hl * bs + bs * (c + 1)],
                            wk_t[:, 128 * c : 128 * (c + 1)],
                            xa,
                            start=False,
                            stop=False,
                        )
                        nc.tensor.matmul(
                            p_v[:, bs * c : bs * (c + 1)],
                            wv_t[:, 128 * c : 128 * (c + 1)],
                            xa,
                            start=(a == 0 and c == 0),
                            stop=False,
                        )
                # rank-1 bias folds close each accumulation group
                for c in range(hl):
                    nc.tensor.matmul(
                        p_qk[:, bs * c : bs * (c + 1)],
                        bq_sb[:, 128 * c : 128 * (c + 1)],
                        onesr[:, :bs],
                        start=False,
                        stop=False,
                    )
                    nc.tensor.matmul(
                        p_qk[:, hl * bs + bs * c : hl * bs + bs * (c + 1)],
                        bk_sb[:, 128 * c : 128 * (c + 1)],
                        onesr[:, :bs],
                        start=False,
                        stop=(c == hl - 1),
                    )
                    nc.tensor.matmul(
                        p_v[:, bs * c : bs * (c + 1)],
                        bv_sb[:, 128 * c : 128 * (c + 1)],
                        onesr[:, :bs],
                        start=False,
                        stop=(c == hl - 1),
                    )
                nc.vector.tensor_copy(qT_sb[:, :], p_qk[:, : hl * bs])
                nc.vector.tensor_copy(kTn_sb[:, :], p_qk[:, hl * bs : 2 * hl * bs])
                nc.vector.tensor_copy(vTn_sb[:, :], p_v[:, :])

            # recover new-token V rows in natural [s, dh] layout per (b, head)
            with tc.tile_pool(name="psT", bufs=2, space="PSUM") as psT:
                for b_ in range(b):
                    for c in range(hl):
                        qcol = c * bs + 4 * b_
                        pT = psT.tile([S, 128], f32, tag="pT")
                        nc.tensor.transpose(
                            pT[:, :], vTn_sb[:, qcol : qcol + 4], ident_sb[:, :]
                        )
                        nc.vector.tensor_copy(
                            vnat_sb[:, (b_ * hl + c) * DH : (b_ * hl + c + 1) * DH],
                            pT[:, :],
                        )

            # ---------------- attention ----------------
            with (
                tc.tile_pool(name="kv", bufs=3) as kvp,
                tc.tile_pool(name="at", bufs=3) as atp,
                tc.tile_pool(name="psA", bufs=2, space="PSUM") as psA,
                tc.tile_pool(name="psB", bufs=2, space="PSUM") as psB,
                tc.tile_pool(name="psD", bufs=2, space="PSUM") as psD,
                tc.tile_pool(name="psN", bufs=2, space="PSUM") as psN,
            ):
                for b_ in range(b):
                    for c in range(hl):
                        bh = b_ * hl + c
                        qcol = c * bs + 4 * b_
                        kT_t = kvp.tile([128, cached], f32, tag="kT")
                        half = cached // 2
                        nc.sync.dma_start(kT_t[:, :half], kT_d[bh, :, :half])
                        nc.sync.dma_start(kT_t[:, half:], kT_d[bh, :, half:])
                        v_t = kvp.tile([128, t2n, DH], f32, tag="v")
                        vv = v_d[bh].rearrange("(a p) dh -> p a dh", p=128)
                        h2 = t2n // 2
                        nc.sync.dma_start(v_t[:, :h2, :], vv[:, :h2, :])
                        nc.sync.dma_start(v_t[:, h2:, :], vv[:, h2:, :])

                        qT_bh = qT_sb[:, qcol : qcol + 4]
                        pA = psA.tile([128, 512], f32, tag="pA")
                        for t2 in range(t2n):
                            nc.tensor.matmul(
                                pA[:, 4 * t2 : 4 * t2 + 4],
                                kT_t[:, 128 * t2 : 128 * (t2 + 1)],
                                qT_bh,
                                start=(t2 == 0),
                                stop=(t2 == t2n - 1),
                            )
                        pN = psN.tile([S, 4], f32, tag="pN")
                        nc.tensor.matmul(
                            pN[:, :],
                            kTn_sb[:, qcol : qcol + 4],
                            qT_bh,
                        )
                        attnT = atp.tile([128, 4 * t2n], f32, tag="attnT")
                        nc.scalar.activation(
                            attnT[:], pA[:, 0 : 4 * t2n], Exp, scale=SCALE
                        )
                        snew = atp.tile([4, 4], f32, tag="snew")
                        nc.vector.tensor_add(snew[:], pN[:, :], mask_sb[:])
                        anew = atp.tile([4, 4], f32, tag="anew")
                        nc.scalar.activation(anew[:], snew[:], Exp, scale=SCALE)
                        # denominators: ones-matmul partials, then reduce
                        pD = psD.tile([128, 4 * t2n + 8], f32, tag="pD")
                        nc.tensor.matmul(
                            pD[0:1, 4 * t2n : 4 * t2n + 4],
                            ones[0:4, :],
                            anew[:, :],
                            start=True,
                            stop=False,
                        )
                        nc.tensor.matmul(
                            pD[0:1, 0 : 4 * t2n],
                            ones[:, :],
                            attnT[:, :],
                            start=False,
                            stop=True,
                        )
                        den = atp.tile([1, 4], f32, tag="den")
                        nc.vector.reduce_sum(
                            den[:],
                            pD[0:1, 0 : 4 * t2n].rearrange(
                                "p (t s) -> p s t", s=4
                            ),
                            axis=AxX,
                        )
                        den2 = atp.tile([1, 4], f32, tag="den2")
                        nc.vector.tensor_add(
                            den2[:], den[:], pD[0:1, 4 * t2n : 4 * t2n + 4]
                        )
                        rden = atp.tile([1, 4], f32, tag="rden")
                        nc.vector.reciprocal(rden[:], den2[:])
                        # ctx accumulation (V stationary, attn moving)
                        pB = psB.tile([128, 8], f32, tag="pB")
                        for t2 in range(t2n):
                            nc.tensor.matmul(
                                pB[:, 0:4],
                                v_t[:, t2, :],
                                attnT[:, 4 * t2 : 4 * t2 + 4],
                                start=(t2 == 0),
                                stop=False,
                            )
                        nc.tensor.matmul(
                            pB[:, 0:4],
                            vnat_sb[:, (b_ * hl + c) * DH : (b_ * hl + c + 1) * DH],
                            anew[:, :],
                            start=False,
                            stop=True,
                        )
                        # broadcast 1/den across partitions (re-uses pD's bank
                        # after the den partials are consumed), then normalize
                        bc0 = 4 * t2n + 4
                        nc.tensor.matmul(
                            pD[:, bc0 : bc0 + 4],
                            onesr[:, :],
                            rden[:, :],
                            start=True,
                            stop=True,
                        )
                        rbc = atp.tile([128, 4], f32, tag="rbc")
                        nc.scalar.copy(rbc[:], pD[:, bc0 : bc0 + 4])
                        nc.vector.tensor_mul(
                            ctxT_sb[:, qcol : qcol + 4], pB[:, 0:4], rbc[:]
                        )

            # ---------------- o_proj (row-parallel partial) ----------------
            with (
                tc.tile_pool(name="wo", bufs=4) as wop,
                tc.tile_pool(name="psO", bufs=2, space="PSUM") as psO,
                tc.tile_pool(name="outp", bufs=1) as outp,
            ):
                out_sb = outp.tile([bs, d], f32)
                for mc in range(mcn):
                    pO = psO.tile([bs, 512], f32, tag="pO")
                    for jcc in range(hl):
                        wo_t = wop.tile([128, 512], f32, tag="wo")
                        nc.sync.dma_start(
                            wo_t[:],
                            woT_d[128 * jcc : 128 * (jcc + 1), 512 * mc : 512 * (mc + 1)],
                        )
                        nc.tensor.matmul(
                            pO[:],
                            ctxT_sb[:, jcc * bs : (jcc + 1) * bs],
                            wo_t[:],
                            start=(jcc == 0),
                            stop=(jcc == hl - 1),
                        )
                    nc.vector.tensor_copy(out_sb[:, 512 * mc : 512 * (mc + 1)], pO[:])
                nc.sync.dma_start(out_d[:, :], out_sb[:])

    if compile_:
        nc.compile()
    return nc


def _mask():
    m = np.zeros((S, S), np.float32)
    m[np.tril_indices(S, -1)] = NEG  # mask[t, s] = NEG where key t > query s
    return m


def make_in_maps(x, k_cache, v_cache, Wq, bq, Wk, bk, Wv, bv, Wo):
    """Shard full inputs into per-core input maps (host-side layout prep)."""
    x = np.asarray(x, np.float32)
    k_cache = np.asarray(k_cache, np.float32)
    v_cache = np.asarray(v_cache, np.float32)
    xT = np.ascontiguousarray(x.reshape(BS, D).T)
    mask = _mask()
    in_maps = []
    for cr in range(NCORES):
        hs = slice(HL * cr, HL * (cr + 1))
        js = slice(JC * cr, JC * (cr + 1))
        kT_c = np.ascontiguousarray(
            k_cache[:, hs].transpose(0, 1, 3, 2)
        ).reshape(B * HL, DH, CACHED)
        v_c = np.ascontiguousarray(v_cache[:, hs]).reshape(B * HL, CACHED, DH)
        in_maps.append(
            {
                "xT": xT,
                "kT": kT_c,
                "v": v_c,
                "wqT": np.ascontiguousarray(np.asarray(Wq, np.float32)[js].T),
                "wkT": np.ascontiguousarray(np.asarray(Wk, np.float32)[js].T),
                "wvT": np.ascontiguousarray(np.asarray(Wv, np.float32)[js].T),
                "woT": np.ascontiguousarray(np.asarray(Wo, np.float32)[:, js].T),
                "bq": np.asarray(bq, np.float32)[js].reshape(1, JC),
                "bk": np.asarray(bk, np.float32)[js].reshape(1, JC),
                "bv": np.asarray(bv, np.float32)[js].reshape(1, JC),
                "mask": mask,
                "ident": np.eye(128, dtype=np.float32),
            }
        )
    return in_maps


def _get_nc():
    if "nc" not in _NC_CACHE:
        _NC_CACHE["nc"] = build_nc()
    return _NC_CACHE["nc"]


def kernel(x, k_cache, v_cache, Wq, bq, Wk, bk, Wv, bv, Wo, bo):
    from concourse.bass_utils import run_bass_kernel_spmd

    nc = _get_nc()
    in_maps = make_in_maps(x, k_cache, v_cache, Wq, bq, Wk, bk, Wv, bv, Wo)
    res = run_bass_kernel_spmd(nc, in_maps, list(range(NCORES)))
    out = np.zeros((BS, D), np.float32)
    for r in res.results:
        out += r["out"]
    out += np.asarray(bo, np.float32)[None, :]
    return out.reshape(B, S, D)


# revision 21
# speedup vs baseline: 1.9810x; 1.9810x over previous
"""Trainium2 Bass kernel for AttentionWithCache (decode-style attention).

Sharding: tensor-parallel over heads. 32 heads split across 8 cores (4 heads
= 512 features per core). Each core projects q/k/v for its heads, attends
over its slice of the KV cache, and computes a partial o_proj (row-parallel);
the host sums the 8 partial outputs (the all-reduce) and adds bo.

V2 design notes (PE-friendly):
  - All heavy matmuls keep the big tensor (K / V cache) as the MOVING
    operand with a fat free dim (512), so the PE array streams at line rate
    and LDWEIGHTS overhead stays off the critical path.
  - K/V stream as bf16 hi/lo pairs (split precision): x ~= hi + lo with both
    halves bf16. A product is computed as hi*hi + hi*lo + lo*hi (3 bf16
    matmuls, fp32 accumulate), dropping only the ~2^-18 lo*lo term; accuracy
    is ~1e-5 relative, 4x faster than native fp32 matmul (4 cyc/row) and
    the same HBM bytes as fp32.
  - Scores are computed in natural [query, token] layout with q stationary.
    The 4 heads' queries are stacked into one stationary operand, but
    compute engines may only address partitions at 32-aligned bases, so
    head h's 4 query rows live at partitions 32h..32h+3 ("spread" layout,
    100 rows); cross-head rows hold garbage that is never read. Matmul
    cost tracks the moving dim, so the wasted rows are free.
  - Softmax denominators fall out of the Exp activation's accum_out.
  - ctx = attn @ V needs attn transposed: attn is tiny, so PE-transpose
    [100,128] chunks, densify + hi/lo-split the 16 useful columns with
    strided-free-dim DVE reads, and accumulate ctx (dense 16 rows) with V
    moving.
  - Normalization (1/den) is deferred to ctx extraction: 1/den is
    transposed to a row, broadcast across partitions with a rank-1 matmul,
    and applied in the final per-head copy.
"""

import sys

if "/opt/trn_rl_repo" not in sys.path:
    sys.path.insert(0, "/opt/trn_rl_repo")

import numpy as np

# Problem constants (hardcoded per contract; kernel.py must be self-contained).
B, S, D, H = 16, 4, 4096, 32
DH = 128
NCORES = 8
HL = H // NCORES          # heads per core = 4
JC = HL * DH              # per-core feature slice = 512
BS = B * S                # 64
CACHED = 4096
TC = 512                  # token streaming chunk
SCALE = float(1.0 / np.sqrt(DH))
NEG = -1.0e30

_NC_CACHE = {}


def build_nc(b=B, hl=HL, d=D, cached=CACHED, debug=False, compile_=True, dump=False):
    """Build (and compile) the per-core Bass program."""
    import concourse.bacc as bacc
    import concourse.mybir as mybir
    import concourse.tile as tile

    f32 = mybir.dt.float32
    bf16 = mybir.dt.bfloat16
    Exp = mybir.ActivationFunctionType.Exp
    AxX = mybir.AxisListType.X

    bs = b * S
    jc = hl * DH
    dt_ = d // 128        # contraction tiles for projections
    mcn = d // 512        # o_proj output chunks
    tcl = min(TC, cached)  # token streaming chunk (shrunk for sim configs)
    assert cached % tcl == 0
    tcn = cached // tcl   # token streaming chunks
    tsn = tcl // 128      # 128-token sub-chunks per streaming chunk
    nr = 4 * hl           # dense stacked rows (head, s) = 16
    sp = 32 * (hl - 1) + 4  # spread rows: head h at partitions 32h..32h+3

    nc = bacc.Bacc("TRN2", target_bir_lowering=False, debug=debug)

    xT_d = nc.dram_tensor("xT", [d, bs], f32, kind="ExternalInput")
    khi_d = nc.dram_tensor("khi", [b, hl, DH, cached], bf16, kind="ExternalInput")
    klo_d = nc.dram_tensor("klo", [b, hl, DH, cached], bf16, kind="ExternalInput")
    vhi_d = nc.dram_tensor("vhi", [b, cached, jc], bf16, kind="ExternalInput")
    vlo_d = nc.dram_tensor("vlo", [b, cached, jc], bf16, kind="ExternalInput")
    wqT_d = nc.dram_tensor("wqT", [d, jc], f32, kind="ExternalInput")
    wkT_d = nc.dram_tensor("wkT", [d, jc], f32, kind="ExternalInput")
    wvT_d = nc.dram_tensor("wvT", [d, jc], f32, kind="ExternalInput")
    woT_d = nc.dram_tensor("woT", [jc, d], f32, kind="ExternalInput")
    bq_d = nc.dram_tensor("bq", [1, jc], f32, kind="ExternalInput")
    bk_d = nc.dram_tensor("bk", [1, jc], f32, kind="ExternalInput")
    bv_d = nc.dram_tensor("bv", [1, jc], f32, kind="ExternalInput")
    mask_d = nc.dram_tensor("mask", [S, S], f32, kind="ExternalInput")
    ident_d = nc.dram_tensor("ident", [128, 128], f32, kind="ExternalInput")
    out_d = nc.dram_tensor("out", [bs, d], f32, kind="ExternalOutput")
    if dump:
        dq_d = nc.dram_tensor("dq", [128, hl, bs], f32, kind="ExternalOutput")
        datt_d = nc.dram_tensor("datt", [sp, cached], f32, kind="ExternalOutput")
        dcta_d = nc.dram_tensor("dcta", [4 * hl, jc], f32, kind="ExternalOutput")
        dctxT_d = nc.dram_tensor("dctxT", [128, hl, bs], f32, kind="ExternalOutput")
        dden_d = nc.dram_tensor("dden", [sp, 1], f32, kind="ExternalOutput")

    with tile.TileContext(nc) as tc:
        with (
            tc.tile_pool(name="persist", bufs=1) as pers,
            tc.tile_pool(name="consts", bufs=1) as cst,
        ):
            onesr = cst.tile([1, 128], f32)
            nc.vector.memset(onesr[:], 1.0)
            mask_sb = cst.tile([S, S], f32)
            nc.sync.dma_start(mask_sb[:], mask_d[:])
            bq_sb = cst.tile([1, jc], f32)
            nc.sync.dma_start(bq_sb[:], bq_d[:])
            bk_sb = cst.tile([1, jc], f32)
            nc.sync.dma_start(bk_sb[:], bk_d[:])
            bv_sb = cst.tile([1, jc], f32)
            nc.sync.dma_start(bv_sb[:], bv_d[:])
            ident_sb = cst.tile([128, 128], f32)
            nc.sync.dma_start(ident_sb[:], ident_d[:])
            xT_sb = cst.tile([128, dt_, bs], f32)
            nc.sync.dma_start(
                xT_sb[:], xT_d[:].rearrange("(a p) i -> p a i", p=128)
            )

            # feature-major projections: [dh, (head, i)] with i = 4*b + s
            qT_sb = pers.tile([128, hl, bs], f32)
            kTn_sb = pers.tile([128, hl, bs], f32)
            vTn_sb = pers.tile([128, hl, bs], f32)
            # spread stationary q: [dh, b, 32h + s] holds q[head h, 4b+s]
            qsp_sb = pers.tile([128, b, 32 * hl], f32)
            qsphi_sb = pers.tile([128, b, 32 * hl], bf16)
            qsplo_sb = pers.tile([128, b, 32 * hl], bf16)
            # new-token V rows in natural [s, dh] layout, per (b, head)
            vnat_sb = pers.tile([S, b * hl * DH], f32)
            ctxT_sb = pers.tile([128, hl, bs], f32)
            # manually ping-ponged attention rows (spread layout); garbage
            # rows must be finite for the transposes, so memset once
            att_pp = [pers.tile([sp, cached], f32, name=f"att{i}") for i in range(2)]
            for t in att_pp:
                nc.vector.memset(t[:], 0.0)
            denp_pp = [pers.tile([sp, tcn], f32, name=f"denp{i}") for i in range(2)]
            dnew_pp = [pers.tile([sp, 1], f32, name=f"dnew{i}") for i in range(2)]
            anew_pp = [pers.tile([sp, S], f32, name=f"anew{i}") for i in range(2)]
            for t in denp_pp + dnew_pp:
                nc.vector.memset(t[:], 1.0)
            for t in anew_pp:
                nc.vector.memset(t[:], 0.0)

            nc.vector.memset(qsp_sb[:], 0.0)
            nc.vector.memset(qsphi_sb[:], 0.0)
            nc.vector.memset(qsplo_sb[:], 0.0)

            # ---------------- projections (fp32, fat moving dim) ----------
            with (
                tc.tile_pool(name="wproj", bufs=4) as wp,
                tc.tile_pool(name="psP", bufs=1, space="PSUM") as psP,
                tc.tile_pool(name="psT0", bufs=2, space="PSUM") as psT0,
            ):
                p_q = psP.tile([bs, jc], f32)
                p_k = psP.tile([bs, jc], f32)
                p_v = psP.tile([bs, jc], f32)
                for a in range(dt_):
                    wq_t = wp.tile([128, jc], f32, tag="w")
                    nc.sync.dma_start(wq_t[:], wqT_d[128 * a : 128 * (a + 1), :])
                    wk_t = wp.tile([128, jc], f32, tag="w")
                    nc.sync.dma_start(wk_t[:], wkT_d[128 * a : 128 * (a + 1), :])
                    wv_t = wp.tile([128, jc], f32, tag="w")
                    nc.sync.dma_start(wv_t[:], wvT_d[128 * a : 128 * (a + 1), :])
                    xa = xT_sb[:, a, :]
                    nc.tensor.matmul(
                        p_q[:, :], xa, wq_t[:, :], start=(a == 0), stop=False
                    )
                    nc.tensor.matmul(
                        p_k[:, :], xa, wk_t[:, :], start=(a == 0), stop=False
                    )
                    nc.tensor.matmul(
                        p_v[:, :], xa, wv_t[:, :], start=(a == 0), stop=False
                    )
                nc.tensor.matmul(
                    p_q[:, :], onesr[:, :bs], bq_sb[:, :], start=False, stop=True
                )
                nc.tensor.matmul(
                    p_k[:, :], onesr[:, :bs], bk_sb[:, :], start=False, stop=True
                )
                nc.tensor.matmul(
                    p_v[:, :], onesr[:, :bs], bv_sb[:, :], start=False, stop=True
                )
                qn_sb = cst.tile([bs, jc], f32)
                nc.vector.tensor_copy(qn_sb[:], p_q[:, :])
                kn_sb = cst.tile([bs, jc], f32)
                nc.vector.tensor_copy(kn_sb[:], p_k[:, :])
                vn_sb = cst.tile([bs, jc], f32)
                nc.vector.tensor_copy(vn_sb[:], p_v[:, :])

                # feature-major transposes of the tiny projection outputs
                for src, dst in ((qn_sb, qT_sb), (kn_sb, kTn_sb), (vn_sb, vTn_sb)):
                    for h in range(hl):
                        pt = psT0.tile([128, bs], f32, tag="pt")
                        nc.tensor.transpose(
                            pt[:, :],
                            src[:, 128 * h : 128 * (h + 1)],
                            ident_sb[:bs, :bs],
                        )
                        nc.vector.tensor_copy(dst[:, h, :], pt[:, :])

                # spread-stationary q: [dh, i, 32h + s] + bf16 hi/lo split
                for h in range(hl):
                    nc.vector.tensor_copy(
                        qsp_sb[:, :, 32 * h : 32 * h + 4],
                        qT_sb[:, h, :].rearrange("p (b4 s) -> p b4 s", s=4),
                    )
                nc.vector.tensor_copy(qsphi_sb[:, :, :], qsp_sb[:, :, :])
                nc.vector.tensor_sub(
                    qsplo_sb[:, :, :], qsp_sb[:, :, :], qsphi_sb[:, :, :]
                )

                # new-token V rows back to natural layout per (b, head)
                for b_ in range(b):
                    for c in range(hl):
                        pt = psT0.tile([S, 128], f32, tag="ptv")
                        nc.tensor.transpose(
                            pt[:, :],
                            vTn_sb[:, c, 4 * b_ : 4 * b_ + 4],
                            ident_sb[:, :],
                        )
                        nc.vector.tensor_copy(
                            vnat_sb[:, (b_ * hl + c) * DH : (b_ * hl + c + 1) * DH],
                            pt[:, :],
                        )

            # ---------------- attention (streaming, split-3 bf16) ---------
            with (
                tc.tile_pool(name="kv", bufs=3) as kvp,
                tc.tile_pool(name="at", bufs=3) as atp,
                tc.tile_pool(name="psS", bufs=2, space="PSUM") as psS,
                tc.tile_pool(name="psC", bufs=2, space="PSUM") as psC,
                tc.tile_pool(name="psT", bufs=2, space="PSUM") as psT,
            ):
                for b_ in range(b):
                    qhi_b = qsphi_sb[:, b_, :sp]
                    qlo_b = qsplo_sb[:, b_, :sp]
                    qf_b = qsp_sb[:, b_, :sp]
                    att_b = att_pp[b_ % 2]
                    denp_b = denp_pp[b_ % 2]
                    dnew_b = dnew_pp[b_ % 2]
                    anew_b = anew_pp[b_ % 2]
                    pC = psC.tile([nr, jc], f32, tag="pC")
                    for tc_ in range(tcn):
                        t0 = tcl * tc_
                        khi_t = kvp.tile([128, hl, tcl], bf16, tag="khi")
                        nc.sync.dma_start(
                            khi_t[:],
                            khi_d[b_, :, :, t0 : t0 + tcl].rearrange(
                                "h p t -> p h t"
                            ),
                        )
                        klo_t = kvp.tile([128, hl, tcl], bf16, tag="klo")
                        nc.sync.dma_start(
                            klo_t[:],
                            klo_d[b_, :, :, t0 : t0 + tcl].rearrange(
                                "h p t -> p h t"
                            ),
                        )
                        vhi_t = kvp.tile([128, tsn, jc], bf16, tag="vhi")
                        nc.sync.dma_start(
                            vhi_t[:],
                            vhi_d[b_, t0 : t0 + tcl, :].rearrange(
                                "(a p) f -> p a f", p=128
                            ),
                        )
                        vlo_t = kvp.tile([128, tsn, jc], bf16, tag="vlo")
                        nc.sync.dma_start(
                            vlo_t[:],
                            vlo_d[b_, t0 : t0 + tcl, :].rearrange(
                                "(a p) f -> p a f", p=128
                            ),
                        )
                        for h in range(hl):
                            pS = psS.tile([sp, tcl], f32, tag="pS")
                            nc.tensor.matmul(
                                pS[:, :], qhi_b, khi_t[:, h, :],
                                start=True, stop=False,
                            )
                            nc.tensor.matmul(
                                pS[:, :], qhi_b, klo_t[:, h, :],
                                start=False, stop=False,
                            )
                            nc.tensor.matmul(
                                pS[:, :], qlo_b, khi_t[:, h, :],
                                start=False, stop=True,
                            )
                            nc.scalar.activation(
                                att_b[32 * h : 32 * h + 4, t0 : t0 + tcl],
                                pS[32 * h : 32 * h + 4, :],
                                Exp,
                                scale=SCALE,
                                accum_out=denp_b[32 * h : 32 * h + 4, tc_ : tc_ + 1],
                            )
                        for ts_ in range(tsn):
                            t2g = tc_ * tsn + ts_
                            pT = psT.tile([128, 32 * hl], f32, tag="pT")
                            nc.tensor.transpose(
                                pT[:, :sp],
                                att_b[:, 128 * t2g : 128 * (t2g + 1)],
                                ident_sb[:sp, :sp],
                            )
                            # densify + hi/lo split the 16 useful columns
                            pTv = pT[:].rearrange(
                                "p (h r) -> p h r", r=32
                            )[:, :, 0:4]
                            athi = atp.tile([128, hl, 4], bf16, tag="athi")
                            nc.vector.tensor_copy(athi[:, :, :], pTv)
                            atlo = atp.tile([128, hl, 4], bf16, tag="atlo")
                            nc.vector.tensor_sub(atlo[:, :, :], pTv, athi[:, :, :])
                            athi_f = athi[:].rearrange("p h s -> p (h s)")
                            atlo_f = atlo[:].rearrange("p h s -> p (h s)")
                            nc.tensor.matmul(
                                pC[:, :], athi_f, vhi_t[:, ts_, :],
                                start=(t2g == 0), stop=False,
                            )
                            nc.tensor.matmul(
                                pC[:, :], athi_f, vlo_t[:, ts_, :],
                                start=False, stop=False,
                            )
                            nc.tensor.matmul(
                                pC[:, :], atlo_f, vhi_t[:, ts_, :],
                                start=False, stop=False,
                            )
                    # new tokens: scores, mask, exp, transpose, ctx
                    pN = psT.tile([sp, 4 * hl], f32, tag="pns", bufs=1)
                    for h in range(hl):
                        nc.tensor.matmul(
                            pN[:, 4 * h : 4 * h + 4],
                            qf_b,
                            kTn_sb[:, h, 4 * b_ : 4 * b_ + 4],
                            start=(h == 0),
                            stop=(h == hl - 1),
                        )
                    snew_b = atp.tile([sp, S], f32, tag="snew")
                    for h in range(hl):
                        nc.vector.tensor_add(
                            snew_b[32 * h : 32 * h + 4, :],
                            pN[32 * h : 32 * h + 4, 4 * h : 4 * h + 4],
                            mask_sb[:, :],
                        )
                        nc.scalar.activation(
                            anew_b[32 * h : 32 * h + 4, :],
                            snew_b[32 * h : 32 * h + 4, :],
                            Exp,
                            scale=SCALE,
                            accum_out=dnew_b[32 * h : 32 * h + 4, :],
                        )
                    pTn = psT.tile([S, 32 * hl], f32, tag="pns", bufs=1)
                    nc.tensor.transpose(
                        pTn[:, :sp], anew_b[:, :], ident_sb[:sp, :sp]
                    )
                    anT_b = atp.tile([S, hl, 4], f32, tag="anT")
                    nc.vector.tensor_copy(
                        anT_b[:, :, :],
                        pTn[:].rearrange("p (h r) -> p h r", r=32)[:, :, 0:4],
                    )
                    nc.tensor.matmul(
                        pC[:, :],
                        anT_b[:].rearrange("p h s -> p (h s)"),
                        vnat_sb[:, (b_ * hl) * DH : (b_ * hl + hl) * DH],
                        start=False,
                        stop=True,
                    )
                    # denominator: reduce, add new, reciprocal, -> dense row
                    dent = atp.tile([sp, 1], f32, tag="dent")
                    nc.vector.reduce_sum(dent[:], denp_b[:], axis=AxX)
                    dent2 = atp.tile([sp, 1], f32, tag="dent2")
                    nc.vector.tensor_add(dent2[:], dent[:], dnew_b[:])
                    rden = atp.tile([sp, 1], f32, tag="rden")
                    nc.vector.reciprocal(rden[:], dent2[:])
                    pTd = psT.tile([1, 32 * hl], f32, tag="pns", bufs=1)
                    nc.tensor.transpose(pTd[:, :sp], rden[:, :], ident_sb[:sp, :sp])
                    rrow = atp.tile([1, hl, 4], f32, tag="rrow")
                    nc.vector.tensor_copy(
                        rrow[:, :, :],
                        pTd[:].rearrange("p (h r) -> p h r", r=32)[:, :, 0:4],
                    )
                    pBC = psT.tile([128, nr], f32, tag="pns", bufs=1)
                    nc.tensor.matmul(
                        pBC[:, :], onesr[:, :], rrow[:].rearrange("p h s -> p (h s)")
                    )
                    rbc = atp.tile([128, nr], f32, tag="rbc")
                    nc.vector.tensor_copy(rbc[:], pBC[:, :])
                    # normalized extraction to feature-major ctxT
                    cta = atp.tile([nr, jc], f32, tag="cta")
                    nc.vector.tensor_copy(cta[:], pC[:, :])
                    for h in range(hl):
                        pT2 = psT.tile([128, nr], f32, tag="pT")
                        nc.tensor.transpose(
                            pT2[:, :],
                            cta[:, 128 * h : 128 * (h + 1)],
                            ident_sb[:nr, :nr],
                        )
                        nc.vector.tensor_mul(
                            ctxT_sb[:, h, 4 * b_ : 4 * b_ + 4],
                            pT2[:, 4 * h : 4 * h + 4],
                            rbc[:, 4 * h : 4 * h + 4],
                        )
                    if dump and b_ == 0:
                        nc.sync.dma_start(datt_d[:, :], att_b[:, :])
                        nc.sync.dma_start(dcta_d[:, :], cta[:])
                        nc.sync.dma_start(dden_d[:, :], dent2[:])

            # ---------------- o_proj (row-parallel partial, fp32) ---------
            with (
                tc.tile_pool(name="wo", bufs=4) as wop,
                tc.tile_pool(name="psO", bufs=2, space="PSUM") as psO,
                tc.tile_pool(name="outp", bufs=1) as outp,
            ):
                out_sb = outp.tile([bs, d], f32)
                for mc in range(mcn):
                    pO = psO.tile([bs, 512], f32, tag="pO")
                    for jcc in range(hl):
                        wo_t = wop.tile([128, 512], f32, tag="wo")
                        nc.sync.dma_start(
                            wo_t[:],
                            woT_d[
                                128 * jcc : 128 * (jcc + 1), 512 * mc : 512 * (mc + 1)
                            ],
                        )
                        nc.tensor.matmul(
                            pO[:],
                            ctxT_sb[:, jcc, :],
                            wo_t[:],
                            start=(jcc == 0),
                            stop=(jcc == hl - 1),
                        )
                    nc.vector.tensor_copy(out_sb[:, 512 * mc : 512 * (mc + 1)], pO[:])
                nc.sync.dma_start(out_d[:, :], out_sb[:])
                if dump:
                    nc.sync.dma_start(dq_d[:, :, :], qT_sb[:, :, :])
                    nc.sync.dma_start(dctxT_d[:, :, :], ctxT_sb[:, :, :])

    if compile_:
        nc.compile()
    return nc


def _mask():
    m = np.zeros((S, S), np.float32)
    m[np.triu_indices(S, 1)] = NEG  # mask[s, t] = NEG where key t > query s
    return m


def _split_bf16(x):
    import ml_dtypes

    hi = x.astype(ml_dtypes.bfloat16)
    lo = (x - hi.astype(np.float32)).astype(ml_dtypes.bfloat16)
    return hi, lo


def make_in_maps(x, k_cache, v_cache, Wq, bq, Wk, bk, Wv, bv, Wo):
    """Shard full inputs into per-core input maps (host-side layout prep)."""
    x = np.asarray(x, np.float32)
    k_cache = np.asarray(k_cache, np.float32)
    v_cache = np.asarray(v_cache, np.float32)
    xT = np.ascontiguousarray(x.reshape(BS, D).T)
    mask = _mask()
    ident = np.eye(128, dtype=np.float32)
    in_maps = []
    for cr in range(NCORES):
        hs = slice(HL * cr, HL * (cr + 1))
        js = slice(JC * cr, JC * (cr + 1))
        # K slice, transposed to [b, h, dh, tok]
        kT_c = np.ascontiguousarray(k_cache[:, hs].transpose(0, 1, 3, 2))
        khi, klo = _split_bf16(kT_c)
        # V slice, heads stacked into features: [b, tok, (h, dh)]
        v_c = np.ascontiguousarray(v_cache[:, hs].transpose(0, 2, 1, 3)).reshape(
            B, CACHED, JC
        )
        vhi, vlo = _split_bf16(v_c)
        in_maps.append(
            {
                "xT": xT,
                "khi": khi,
                "klo": klo,
                "vhi": vhi,
                "vlo": vlo,
                "wqT": np.ascontiguousarray(np.asarray(Wq, np.float32)[js].T),
                "wkT": np.ascontiguousarray(np.asarray(Wk, np.float32)[js].T),
                "wvT": np.ascontiguousarray(np.asarray(Wv, np.float32)[js].T),
                "woT": np.ascontiguousarray(np.asarray(Wo, np.float32)[:, js].T),
                "bq": np.asarray(bq, np.float32)[js].reshape(1, JC),
                "bk": np.asarray(bk, np.float32)[js].reshape(1, JC),
                "bv": np.asarray(bv, np.float32)[js].reshape(1, JC),
                "mask": mask,
                "ident": ident,
            }
        )
    return in_maps


def _get_nc():
    if "nc" not in _NC_CACHE:
        _NC_CACHE["nc"] = build_nc()
    return _NC_CACHE["nc"]


def kernel(x, k_cache, v_cache, Wq, bq, Wk, bk, Wv, bv, Wo, bo):
    from concourse.bass_utils import run_bass_kernel_spmd

    nc = _get_nc()
    in_maps = make_in_maps(x, k_cache, v_cache, Wq, bq, Wk, bk, Wv, bv, Wo)
    res = run_bass_kernel_spmd(nc, in_maps, list(range(NCORES)))
    out = np.zeros((BS, D), np.float32)
    for r in res.results:
        out += r["out"]
    out += np.asarray(bo, np.float32)[None, :]
    return out.reshape(B, S, D)


# revision 23
# speedup vs baseline: 2.0686x; 1.0442x over previous
"""Trainium2 Bass kernel for AttentionWithCache (decode-style attention).

Sharding: tensor-parallel over heads. 32 heads split across 8 cores (4 heads
= 512 features per core). Each core projects q/k/v for its heads, attends
over its slice of the KV cache, and computes a partial o_proj (row-parallel);
the host sums the 8 partial outputs (the all-reduce) and adds bo.

V2 design notes (PE-friendly):
  - All heavy matmuls keep the big tensor (K / V cache) as the MOVING
    operand with a fat free dim (512), so the PE array streams at line rate
    and LDWEIGHTS overhead stays off the critical path.
  - K/V stream as bf16 hi/lo pairs (split precision): x ~= hi + lo with both
    halves bf16. A product is computed as hi*hi + hi*lo + lo*hi (3 bf16
    matmuls, fp32 accumulate), dropping only the ~2^-18 lo*lo term; accuracy
    is ~1e-5 relative, 4x faster than native fp32 matmul (4 cyc/row) and
    the same HBM bytes as fp32.
  - Scores are computed in natural [query, token] layout with q stationary.
    The 4 heads' queries are stacked into one stationary operand, but
    compute engines may only address partitions at 32-aligned bases, so
    head h's 4 query rows live at partitions 32h..32h+3 ("spread" layout,
    100 rows); cross-head rows hold garbage that is never read. Matmul
    cost tracks the moving dim, so the wasted rows are free.
  - Softmax denominators fall out of the Exp activation's accum_out.
  - ctx = attn @ V needs attn transposed: attn is tiny, so PE-transpose
    [100,128] chunks, densify + hi/lo-split the 16 useful columns with
    strided-free-dim DVE reads, and accumulate ctx (dense 16 rows) with V
    moving.
  - Normalization (1/den) is deferred to ctx extraction: 1/den is
    transposed to a row, broadcast across partitions with a rank-1 matmul,
    and applied in the final per-head copy.
"""

import sys

if "/opt/trn_rl_repo" not in sys.path:
    sys.path.insert(0, "/opt/trn_rl_repo")

import numpy as np

# Problem constants (hardcoded per contract; kernel.py must be self-contained).
B, S, D, H = 16, 4, 4096, 32
DH = 128
NCORES = 8
HL = H // NCORES          # heads per core = 4
JC = HL * DH              # per-core feature slice = 512
BS = B * S                # 64
CACHED = 4096
TC = 512                  # token streaming chunk
SCALE = float(1.0 / np.sqrt(DH))
NEG = -1.0e30

_NC_CACHE = {}


def build_nc(b=B, hl=HL, d=D, cached=CACHED, debug=False, compile_=True, dump=False):
    """Build (and compile) the per-core Bass program."""
    import concourse.bacc as bacc
    import concourse.mybir as mybir
    import concourse.tile as tile

    f32 = mybir.dt.float32
    bf16 = mybir.dt.bfloat16
    Exp = mybir.ActivationFunctionType.Exp
    AxX = mybir.AxisListType.X

    bs = b * S
    jc = hl * DH
    dt_ = d // 128        # contraction tiles for projections
    mcn = d // 512        # o_proj output chunks
    tcl = min(TC, cached)       # score-matmul token chunk (<=512)
    kcl = min(2 * TC, cached)   # K DMA chunk
    vcl = min(2 * TC, cached)   # V DMA chunk
    assert cached % kcl == 0 and cached % vcl == 0
    kcn = cached // kcl
    vcn = cached // vcl
    scn = kcl // tcl            # score chunks per K chunk
    vsn = vcl // 128            # 128-token sub-chunks per V chunk
    nr = 4 * hl           # dense stacked rows (head, s) = 16
    sp = 32 * (hl - 1) + 4  # spread rows: head h at partitions 32h..32h+3

    nc = bacc.Bacc("TRN2", target_bir_lowering=False, debug=debug)

    xT_d = nc.dram_tensor("xT", [d, bs], f32, kind="ExternalInput")
    khi_d = nc.dram_tensor("khi", [b, hl, DH, cached], bf16, kind="ExternalInput")
    klo_d = nc.dram_tensor("klo", [b, hl, DH, cached], bf16, kind="ExternalInput")
    vhi_d = nc.dram_tensor("vhi", [b, 128, cached // 128, jc], bf16, kind="ExternalInput")
    vlo_d = nc.dram_tensor("vlo", [b, 128, cached // 128, jc], bf16, kind="ExternalInput")
    wqT_d = nc.dram_tensor("wqT", [d, jc], f32, kind="ExternalInput")
    wkT_d = nc.dram_tensor("wkT", [d, jc], f32, kind="ExternalInput")
    wvT_d = nc.dram_tensor("wvT", [d, jc], f32, kind="ExternalInput")
    woT_d = nc.dram_tensor("woT", [jc, d], f32, kind="ExternalInput")
    bq_d = nc.dram_tensor("bq", [1, jc], f32, kind="ExternalInput")
    bk_d = nc.dram_tensor("bk", [1, jc], f32, kind="ExternalInput")
    bv_d = nc.dram_tensor("bv", [1, jc], f32, kind="ExternalInput")
    mask_d = nc.dram_tensor("mask", [S, S], f32, kind="ExternalInput")
    ident_d = nc.dram_tensor("ident", [128, 128], f32, kind="ExternalInput")
    out_d = nc.dram_tensor("out", [bs, d], f32, kind="ExternalOutput")
    if dump:
        dq_d = nc.dram_tensor("dq", [128, hl, bs], f32, kind="ExternalOutput")
        datt_d = nc.dram_tensor("datt", [sp, cached], f32, kind="ExternalOutput")
        dcta_d = nc.dram_tensor("dcta", [4 * hl, jc], f32, kind="ExternalOutput")
        dctxT_d = nc.dram_tensor("dctxT", [128, hl, bs], f32, kind="ExternalOutput")
        dden_d = nc.dram_tensor("dden", [sp, 1], f32, kind="ExternalOutput")

    with tile.TileContext(nc) as tc:
        with (
            tc.tile_pool(name="persist", bufs=1) as pers,
            tc.tile_pool(name="consts", bufs=1) as cst,
        ):
            onesr = cst.tile([1, 128], f32)
            nc.vector.memset(onesr[:], 1.0)
            mask_sb = cst.tile([S, S], f32)
            nc.sync.dma_start(mask_sb[:], mask_d[:])
            bq_sb = cst.tile([1, jc], f32)
            nc.sync.dma_start(bq_sb[:], bq_d[:])
            bk_sb = cst.tile([1, jc], f32)
            nc.sync.dma_start(bk_sb[:], bk_d[:])
            bv_sb = cst.tile([1, jc], f32)
            nc.sync.dma_start(bv_sb[:], bv_d[:])
            ident_sb = cst.tile([128, 128], f32)
            nc.sync.dma_start(ident_sb[:], ident_d[:])
            # feature-major projections: [dh, (head, i)] with i = 4*b + s
            qT_sb = pers.tile([128, hl, bs], f32)
            kTn_sb = pers.tile([128, hl, bs], f32)
            vTn_sb = pers.tile([128, hl, bs], f32)
            # spread stationary q: [dh, b, 32h + s] holds q[head h, 4b+s]
            qsp_sb = pers.tile([128, b, 32 * hl], f32)
            qsphi_sb = pers.tile([128, b, 32 * hl], bf16)
            qsplo_sb = pers.tile([128, b, 32 * hl], bf16)
            ctxT_sb = pers.tile([128, hl, bs], f32)
            # manually ping-ponged attention rows (spread layout); garbage
            # rows must be finite for the transposes, so memset once
            att_pp = [pers.tile([sp, cached], f32, name=f"att{i}") for i in range(2)]
            for t in att_pp:
                nc.vector.memset(t[:], 0.0)
            denp_pp = [pers.tile([sp, kcn], f32, name=f"denp{i}") for i in range(2)]
            dnew_pp = [pers.tile([sp, 1], f32, name=f"dnew{i}") for i in range(2)]
            anew_pp = [pers.tile([sp, S], f32, name=f"anew{i}") for i in range(2)]
            for t in denp_pp + dnew_pp:
                nc.vector.memset(t[:], 1.0)
            for t in anew_pp:
                nc.vector.memset(t[:], 0.0)

            nc.vector.memset(qsp_sb[:], 0.0)
            nc.vector.memset(qsphi_sb[:], 0.0)
            nc.vector.memset(qsplo_sb[:], 0.0)

            # ---------------- projections (fp32, fat moving dim) ----------
            with (
                tc.tile_pool(name="wproj", bufs=4) as wp,
                tc.tile_pool(name="psP", bufs=1, space="PSUM") as psP,
                tc.tile_pool(name="psT0", bufs=2, space="PSUM") as psT0,
            ):
                xT_sb = wp.tile([128, dt_, bs], f32, tag="xT", bufs=1)
                nc.sync.dma_start(
                    xT_sb[:], xT_d[:].rearrange("(a p) i -> p a i", p=128)
                )
                p_q = psP.tile([bs, jc], f32)
                p_k = psP.tile([bs, jc], f32)
                p_v = psP.tile([bs, jc], f32)
                for a in range(dt_):
                    wq_t = wp.tile([128, jc], f32, tag="w")
                    nc.sync.dma_start(wq_t[:], wqT_d[128 * a : 128 * (a + 1), :])
                    wk_t = wp.tile([128, jc], f32, tag="w")
                    nc.sync.dma_start(wk_t[:], wkT_d[128 * a : 128 * (a + 1), :])
                    wv_t = wp.tile([128, jc], f32, tag="w")
                    nc.sync.dma_start(wv_t[:], wvT_d[128 * a : 128 * (a + 1), :])
                    xa = xT_sb[:, a, :]
                    nc.tensor.matmul(
                        p_q[:, :], xa, wq_t[:, :], start=(a == 0), stop=False
                    )
                    nc.tensor.matmul(
                        p_k[:, :], xa, wk_t[:, :], start=(a == 0), stop=False
                    )
                    nc.tensor.matmul(
                        p_v[:, :], xa, wv_t[:, :], start=(a == 0), stop=False
                    )
                nc.tensor.matmul(
                    p_q[:, :], onesr[:, :bs], bq_sb[:, :], start=False, stop=True
                )
                nc.tensor.matmul(
                    p_k[:, :], onesr[:, :bs], bk_sb[:, :], start=False, stop=True
                )
                nc.tensor.matmul(
                    p_v[:, :], onesr[:, :bs], bv_sb[:, :], start=False, stop=True
                )
                qn_sb = wp.tile([bs, jc], f32, tag="qn", bufs=1)
                nc.vector.tensor_copy(qn_sb[:], p_q[:, :])
                kn_sb = wp.tile([bs, jc], f32, tag="kn", bufs=1)
                nc.vector.tensor_copy(kn_sb[:], p_k[:, :])
                vn_sb = wp.tile([bs, jc], f32, tag="vn", bufs=1)
                nc.vector.tensor_copy(vn_sb[:], p_v[:, :])

                # feature-major transposes of the tiny projection outputs
                for src, dst in ((qn_sb, qT_sb), (kn_sb, kTn_sb), (vn_sb, vTn_sb)):
                    for h in range(hl):
                        pt = psT0.tile([128, bs], f32, tag="pt")
                        nc.tensor.transpose(
                            pt[:, :],
                            src[:, 128 * h : 128 * (h + 1)],
                            ident_sb[:bs, :bs],
                        )
                        nc.vector.tensor_copy(dst[:, h, :], pt[:, :])

                # spread-stationary q: [dh, i, 32h + s] + bf16 hi/lo split
                for h in range(hl):
                    nc.vector.tensor_copy(
                        qsp_sb[:, :, 32 * h : 32 * h + 4],
                        qT_sb[:, h, :].rearrange("p (b4 s) -> p b4 s", s=4),
                    )
                nc.vector.tensor_copy(qsphi_sb[:, :, :], qsp_sb[:, :, :])
                nc.vector.tensor_sub(
                    qsplo_sb[:, :, :], qsp_sb[:, :, :], qsphi_sb[:, :, :]
                )


            # ---------------- attention (streaming, split-3 bf16) ---------
            with (
                tc.tile_pool(name="kv", bufs=3) as kvp,
                tc.tile_pool(name="at", bufs=3) as atp,
                tc.tile_pool(name="psS", bufs=2, space="PSUM") as psS,
                tc.tile_pool(name="psC", bufs=2, space="PSUM") as psC,
                tc.tile_pool(name="psT", bufs=2, space="PSUM") as psT,
            ):
                for b_ in range(b):
                    qhi_b = qsphi_sb[:, b_, :sp]
                    qlo_b = qsplo_sb[:, b_, :sp]
                    qf_b = qsp_sb[:, b_, :sp]
                    att_b = att_pp[b_ % 2]
                    denp_b = denp_pp[b_ % 2]
                    dnew_b = dnew_pp[b_ % 2]
                    anew_b = anew_pp[b_ % 2]
                    pC = psC.tile([nr, jc], f32, tag="pC")
                    for kc in range(kcn):
                        k0 = kcl * kc
                        khi_t = kvp.tile([128, hl, kcl], bf16, tag="khi")
                        nc.sync.dma_start(
                            khi_t[:],
                            khi_d[b_, :, :, k0 : k0 + kcl].rearrange(
                                "h p t -> p h t"
                            ),
                        )
                        klo_t = kvp.tile([128, hl, kcl], bf16, tag="klo")
                        nc.sync.dma_start(
                            klo_t[:],
                            klo_d[b_, :, :, k0 : k0 + kcl].rearrange(
                                "h p t -> p h t"
                            ),
                        )
                        for sc_ in range(scn):
                            s0 = tcl * sc_
                            t0 = k0 + s0
                            for h in range(hl):
                                pS = psS.tile([sp, tcl], f32, tag="pS")
                                nc.tensor.matmul(
                                    pS[:, :], qhi_b,
                                    khi_t[:, h, s0 : s0 + tcl],
                                    start=True, stop=False,
                                )
                                nc.tensor.matmul(
                                    pS[:, :], qhi_b,
                                    klo_t[:, h, s0 : s0 + tcl],
                                    start=False, stop=False,
                                )
                                nc.tensor.matmul(
                                    pS[:, :], qlo_b,
                                    khi_t[:, h, s0 : s0 + tcl],
                                    start=False, stop=True,
                                )
                                nc.scalar.activation(
                                    att_b[32 * h : 32 * h + 4, t0 : t0 + tcl],
                                    pS[32 * h : 32 * h + 4, :],
                                    Exp,
                                    scale=SCALE,
                                )
                        # denominator partials on the DVE (free-dim reduce)
                        nc.vector.reduce_sum(
                            denp_b[:, kc : kc + 1],
                            att_b[:, k0 : k0 + kcl],
                            axis=AxX,
                        )
                    for vc_ in range(vcn):
                        v0 = vcl * vc_
                        a0 = v0 // 128
                        vhi_t = kvp.tile([128, vsn, jc], bf16, tag="vhi")
                        nc.scalar.dma_start(
                            vhi_t[:], vhi_d[b_, :, a0 : a0 + vsn, :]
                        )
                        vlo_t = kvp.tile([128, vsn, jc], bf16, tag="vlo")
                        nc.scalar.dma_start(
                            vlo_t[:], vlo_d[b_, :, a0 : a0 + vsn, :]
                        )
                        for ts_ in range(vsn):
                            t2g = vc_ * vsn + ts_
                            pT = psT.tile([128, 32 * hl], f32, tag="pT")
                            nc.tensor.transpose(
                                pT[:, :sp],
                                att_b[:, 128 * t2g : 128 * (t2g + 1)],
                                ident_sb[:sp, :sp],
                            )
                            # densify + hi/lo split the 16 useful columns
                            pTv = pT[:].rearrange(
                                "p (h r) -> p h r", r=32
                            )[:, :, 0:4]
                            athi = atp.tile([128, hl, 4], bf16, tag="athi")
                            nc.vector.tensor_copy(athi[:, :, :], pTv)
                            atlo = atp.tile([128, hl, 4], bf16, tag="atlo")
                            nc.vector.tensor_sub(atlo[:, :, :], pTv, athi[:, :, :])
                            athi_f = athi[:].rearrange("p h s -> p (h s)")
                            atlo_f = atlo[:].rearrange("p h s -> p (h s)")
                            nc.tensor.matmul(
                                pC[:, :], athi_f, vhi_t[:, ts_, :],
                                start=(t2g == 0), stop=False,
                            )
                            nc.tensor.matmul(
                                pC[:, :], athi_f, vlo_t[:, ts_, :],
                                start=False, stop=False,
                            )
                            nc.tensor.matmul(
                                pC[:, :], atlo_f, vhi_t[:, ts_, :],
                                start=False, stop=False,
                            )
                    # new tokens: scores, mask, exp, transpose, ctx
                    pN = psT.tile([sp, 4 * hl], f32, tag="pns", bufs=1)
                    for h in range(hl):
                        nc.tensor.matmul(
                            pN[:, 4 * h : 4 * h + 4],
                            qf_b,
                            kTn_sb[:, h, 4 * b_ : 4 * b_ + 4],
                            start=(h == 0),
                            stop=(h == hl - 1),
                        )
                    snew_b = atp.tile([sp, S], f32, tag="snew")
                    for h in range(hl):
                        nc.vector.tensor_add(
                            snew_b[32 * h : 32 * h + 4, :],
                            pN[32 * h : 32 * h + 4, 4 * h : 4 * h + 4],
                            mask_sb[:, :],
                        )
                        nc.scalar.activation(
                            anew_b[32 * h : 32 * h + 4, :],
                            snew_b[32 * h : 32 * h + 4, :],
                            Exp,
                            scale=SCALE,
                            accum_out=dnew_b[32 * h : 32 * h + 4, :],
                        )
                    pTn = psT.tile([S, 32 * hl], f32, tag="pns", bufs=1)
                    nc.tensor.transpose(
                        pTn[:, :sp], anew_b[:, :], ident_sb[:sp, :sp]
                    )
                    anT_b = atp.tile([S, hl, 4], f32, tag="anT")
                    nc.vector.tensor_copy(
                        anT_b[:, :, :],
                        pTn[:].rearrange("p (h r) -> p h r", r=32)[:, :, 0:4],
                    )
                    vnat_b = atp.tile([S, jc], f32, tag="vnat")
                    for c in range(hl):
                        ptv = psT.tile([S, 128], f32, tag="pns", bufs=1)
                        nc.tensor.transpose(
                            ptv[:, :],
                            vTn_sb[:, c, 4 * b_ : 4 * b_ + 4],
                            ident_sb[:, :],
                        )
                        nc.vector.tensor_copy(
                            vnat_b[:, 128 * c : 128 * (c + 1)], ptv[:, :]
                        )
                    nc.tensor.matmul(
                        pC[:, :],
                        anT_b[:].rearrange("p h s -> p (h s)"),
                        vnat_b[:],
                        start=False,
                        stop=True,
                    )
                    # denominator: reduce, add new, reciprocal, -> dense row
                    dent = atp.tile([sp, 1], f32, tag="dent")
                    nc.vector.reduce_sum(dent[:], denp_b[:], axis=AxX)
                    dent2 = atp.tile([sp, 1], f32, tag="dent2")
                    nc.vector.tensor_add(dent2[:], dent[:], dnew_b[:])
                    rden = atp.tile([sp, 1], f32, tag="rden")
                    nc.vector.reciprocal(rden[:], dent2[:])
                    pTd = psT.tile([1, 32 * hl], f32, tag="pns", bufs=1)
                    nc.tensor.transpose(pTd[:, :sp], rden[:, :], ident_sb[:sp, :sp])
                    rrow = atp.tile([1, hl, 4], f32, tag="rrow")
                    nc.vector.tensor_copy(
                        rrow[:, :, :],
                        pTd[:].rearrange("p (h r) -> p h r", r=32)[:, :, 0:4],
                    )
                    pBC = psT.tile([128, nr], f32, tag="pns", bufs=1)
                    nc.tensor.matmul(
                        pBC[:, :], onesr[:, :], rrow[:].rearrange("p h s -> p (h s)")
                    )
                    rbc = atp.tile([128, nr], f32, tag="rbc")
                    nc.vector.tensor_copy(rbc[:], pBC[:, :])
                    # normalized extraction to feature-major ctxT
                    cta = atp.tile([nr, jc], f32, tag="cta")
                    nc.vector.tensor_copy(cta[:], pC[:, :])
                    for h in range(hl):
                        pT2 = psT.tile([128, nr], f32, tag="pT")
                        nc.tensor.transpose(
                            pT2[:, :],
                            cta[:, 128 * h : 128 * (h + 1)],
                            ident_sb[:nr, :nr],
                        )
                        nc.vector.tensor_mul(
                            ctxT_sb[:, h, 4 * b_ : 4 * b_ + 4],
                            pT2[:, 4 * h : 4 * h + 4],
                            rbc[:, 4 * h : 4 * h + 4],
                        )
                    if dump and b_ == 0:
                        nc.sync.dma_start(datt_d[:, :], att_b[:, :])
                        nc.sync.dma_start(dcta_d[:, :], cta[:])
                        nc.sync.dma_start(dden_d[:, :], dent2[:])

            # ---------------- o_proj (row-parallel partial, fp32) ---------
            with (
                tc.tile_pool(name="wo", bufs=4) as wop,
                tc.tile_pool(name="psO", bufs=2, space="PSUM") as psO,
                tc.tile_pool(name="outp", bufs=3) as outp,
            ):
                for mc in range(mcn):
                    pO = psO.tile([bs, 512], f32, tag="pO")
                    for jcc in range(hl):
                        wo_t = wop.tile([128, 512], f32, tag="wo")
                        nc.sync.dma_start(
                            wo_t[:],
                            woT_d[
                                128 * jcc : 128 * (jcc + 1), 512 * mc : 512 * (mc + 1)
                            ],
                        )
                        nc.tensor.matmul(
                            pO[:],
                            ctxT_sb[:, jcc, :],
                            wo_t[:],
                            start=(jcc == 0),
                            stop=(jcc == hl - 1),
                        )
                    ot = outp.tile([bs, 512], f32, tag="ot")
                    nc.vector.tensor_copy(ot[:], pO[:])
                    nc.scalar.dma_start(
                        out_d[:, 512 * mc : 512 * (mc + 1)], ot[:]
                    )
                if dump:
                    nc.sync.dma_start(dq_d[:, :, :], qT_sb[:, :, :])
                    nc.sync.dma_start(dctxT_d[:, :, :], ctxT_sb[:, :, :])

    if compile_:
        nc.compile()
    return nc


def _mask():
    m = np.zeros((S, S), np.float32)
    m[np.triu_indices(S, 1)] = NEG  # mask[s, t] = NEG where key t > query s
    return m


def _split_bf16(x):
    import ml_dtypes

    hi = x.astype(ml_dtypes.bfloat16)
    lo = (x - hi.astype(np.float32)).astype(ml_dtypes.bfloat16)
    return hi, lo


def make_in_maps(x, k_cache, v_cache, Wq, bq, Wk, bk, Wv, bv, Wo):
    """Shard full inputs into per-core input maps (host-side layout prep)."""
    x = np.asarray(x, np.float32)
    k_cache = np.asarray(k_cache, np.float32)
    v_cache = np.asarray(v_cache, np.float32)
    xT = np.ascontiguousarray(x.reshape(BS, D).T)
    mask = _mask()
    ident = np.eye(128, dtype=np.float32)
    in_maps = []
    for cr in range(NCORES):
        hs = slice(HL * cr, HL * (cr + 1))
        js = slice(JC * cr, JC * (cr + 1))
        # K slice, transposed to [b, h, dh, tok]
        kT_c = np.ascontiguousarray(k_cache[:, hs].transpose(0, 1, 3, 2))
        khi, klo = _split_bf16(kT_c)
        # V slice, heads stacked into features: [b, tok, (h, dh)]
        v_c = np.ascontiguousarray(v_cache[:, hs].transpose(0, 2, 1, 3)).reshape(
            B, CACHED, JC
        )
        vhi, vlo = _split_bf16(v_c)
        vhi = np.ascontiguousarray(
            vhi.reshape(B, CACHED // 128, 128, JC).transpose(0, 2, 1, 3)
        )
        vlo = np.ascontiguousarray(
            vlo.reshape(B, CACHED // 128, 128, JC).transpose(0, 2, 1, 3)
        )
        in_maps.append(
            {
                "xT": xT,
                "khi": khi,
                "klo": klo,
                "vhi": vhi,
                "vlo": vlo,
                "wqT": np.ascontiguousarray(np.asarray(Wq, np.float32)[js].T),
                "wkT": np.ascontiguousarray(np.asarray(Wk, np.float32)[js].T),
                "wvT": np.ascontiguousarray(np.asarray(Wv, np.float32)[js].T),
                "woT": np.ascontiguousarray(np.asarray(Wo, np.float32)[:, js].T),
                "bq": np.asarray(bq, np.float32)[js].reshape(1, JC),
                "bk": np.asarray(bk, np.float32)[js].reshape(1, JC),
                "bv": np.asarray(bv, np.float32)[js].reshape(1, JC),
                "mask": mask,
                "ident": ident,
            }
        )
    return in_maps


def _get_nc():
    if "nc" not in _NC_CACHE:
        _NC_CACHE["nc"] = build_nc()
    return _NC_CACHE["nc"]


def kernel(x, k_cache, v_cache, Wq, bq, Wk, bk, Wv, bv, Wo, bo):
    from concourse.bass_utils import run_bass_kernel_spmd

    nc = _get_nc()
    in_maps = make_in_maps(x, k_cache, v_cache, Wq, bq, Wk, bk, Wv, bv, Wo)
    res = run_bass_kernel_spmd(nc, in_maps, list(range(NCORES)))
    out = np.zeros((BS, D), np.float32)
    for r in res.results:
        out += r["out"]
    out += np.asarray(bo, np.float32)[None, :]
    return out.reshape(B, S, D)
